# revision 1
# baseline (speedup 1.0000x reference)
"""Trainium2 Bass kernel for CrossModalAttention.

Reference computation (per modality pair):
    ctx_a = softmax((Wq xa)^T (Wk xb) * rdim^-0.5) applied to (Wv xb)
    enh_a = BatchNorm(xa + ctx_a)   # training-mode BN, stats over (B, H, W)

Sharding: 8 cores <- 8 independent (batch, modality) attention problems
(B=4 x 2 modalities).  Core b handles (batch b, wli), core 4+b handles
(batch b, nbi).  BatchNorm stats are synced with a tiny AllReduce across
the 4 cores of each modality: replica_groups=[[0,1,2,3],[4,5,6,7]]
(contiguous groups -- strided groups measured pathologically slow).

Per-core kernel layout choices:
  - scores are computed TRANSPOSED (keys on partitions, queries on free dim)
    so softmax needs no max/transpose passes: exp is elementwise and the
    denominator falls out of the attn@v matmul via a ones-column appended
    to v^T (free dim 257 = 256 channels + 1 ones).
  - all PE matmuls run in bf16 with fp32 PSUM accumulation (fp32r needs
    producers that round to fp32r, which walrus rejects for DMA'd inputs);
    inputs are cast to bf16 shadows once on the ACT engine.
  - scores^T matmuls have K=64, so two key-chunks are packed onto the PE
    concurrently via row tiling (chunk j on rows 0..63, chunk j+16 on rows
    64..127).
  - ctx^T (pixels x channels) is PE-transposed back to channel-major in
    128x128 tiles, then residual-add + BN happen in natural layout.
  - the main loop is software-pipelined: scores/exp for tile t+1 are
    emitted before attn@v of tile t so the PE never stalls on the ACT
    engine's exp throughput.
"""

import sys
from contextlib import ExitStack

import numpy as np

if "/opt/trn_rl_repo" not in sys.path:
    sys.path.insert(0, "/opt/trn_rl_repo")

import concourse.bass as bass  # noqa: F401
import concourse.mybir as mybir
import concourse.tile as tile
from concourse import bacc
from concourse.bass_utils import run_bass_kernel_spmd
from concourse.masks import make_identity

F32 = mybir.dt.float32
F32R = mybir.dt.float32r
BF16 = mybir.dt.bfloat16

DIM = 256          # channels
RDIM = 64          # attention head dim
H = W = 64
N = H * W          # 4096 pixels
B = 4
NCORES = 8
SCALE = RDIM ** -0.5   # 0.125
EPS = 1e-5
NSTAT = B * N      # BN sample count per channel (over all batches)

P = 128            # partitions
CCH = DIM // P     # 2 channel chunks
NQT = 512          # query tile (free dim of scores^T matmul)
NT = N // NQT      # 8 query tiles
NKC = N // P       # 32 key chunks of 128
NPAIR = NKC // 2   # 16 row-tiled score chunk pairs
NQC = NQT // P     # 4 query sub-chunks per tile
VTW = DIM + 1      # 257 = v channels + ones column

_CACHE = {}
NUM_DEVICES = NCORES  # 1 for single-core TimelineSim profiling
USE_COLLECTIVE = True
NT_RUN = NT          # how many attention tiles to emit (bisect knob)
RUN_PROJ = True      # emit projection phase
REPEAT = 1           # repeat compute phases (timing amplification, unrolled)
LOOP_R = 0           # >0: wrap proj+attention in a tc.For_i runtime loop of this
                     # many iterations (constant NEFF size -- used for timing)




def _build_program():
    nc = bacc.Bacc(
        "TRN2",
        target_bir_lowering=False,
        debug=False,
        enable_asserts=False,
        num_devices=NUM_DEVICES,
    )

    xq = nc.dram_tensor("xq", [DIM, N], F32, kind="ExternalInput").ap()
    xkv = nc.dram_tensor("xkv", [DIM, N], F32, kind="ExternalInput").ap()
    wq = nc.dram_tensor("wq", [RDIM, DIM], F32, kind="ExternalInput").ap()
    wk = nc.dram_tensor("wk", [RDIM, DIM], F32, kind="ExternalInput").ap()
    wv = nc.dram_tensor("wv", [DIM, DIM], F32, kind="ExternalInput").ap()
    gamma = nc.dram_tensor("gamma", [DIM], F32, kind="ExternalInput").ap()
    beta = nc.dram_tensor("beta", [DIM], F32, kind="ExternalInput").ap()
    out = nc.dram_tensor("out", [DIM, N], F32, kind="ExternalOutput").ap()

    with tile.TileContext(nc) as tc:
        _body(tc, xq, xkv, wq, wk, wv, gamma, beta, out)

    nc.compile()
    return nc


def _body(tc, xq, xkv, wq, wk, wv, gamma, beta, out):
    nc = tc.nc
    Exp = mybir.ActivationFunctionType.Exp
    Sqrt = mybir.ActivationFunctionType.Sqrt
    add = mybir.AluOpType.add
    mult = mybir.AluOpType.mult
    subtract = mybir.AluOpType.subtract
    AX = mybir.AxisListType.X

    ctx = ExitStack()
    with ctx:
        con = ctx.enter_context(tc.tile_pool(name="con", bufs=1))
        expp = ctx.enter_context(tc.tile_pool(name="expp", bufs=2))
        ctxp = ctx.enter_context(tc.tile_pool(name="ctxp", bufs=3))
        sml = ctx.enter_context(tc.tile_pool(name="sml", bufs=4))
        dram = ctx.enter_context(tc.tile_pool(name="dram", bufs=1, space="DRAM"))
        ps_s = ctx.enter_context(tc.tile_pool(name="ps_s", bufs=2, space="PSUM"))
        ps_o = ctx.enter_context(tc.tile_pool(name="ps_o", bufs=2, space="PSUM"))
        ps_t = ctx.enter_context(tc.tile_pool(name="ps_t", bufs=2, space="PSUM"))

        # ---- persistent SBUF tensors ----
        xq_sb = con.tile([P, CCH * N], F32, name="xq_sb")    # [c%128, cchunk*N + pix]
        xkv_sb = con.tile([P, CCH * N], F32, name="xkv_sb")
        xq_b = con.tile([P, CCH * N], BF16, name="xq_b")
        xkv_b = con.tile([P, CCH * N], BF16, name="xkv_b")
        qf = con.tile([P, N], BF16, name="qf")                # q duplicated on both halves
        kf2 = con.tile([P, NPAIR * P], BF16, name="kf2")      # k: chunk j top, chunk j+16 bottom
        vt = con.tile([P, NKC * VTW], BF16, name="vt")       # v^T tiles + ones column
        wq_raw = con.tile([RDIM, DIM], F32, name="wq_raw")
        wk_raw = con.tile([RDIM, DIM], F32, name="wk_raw")
        wv_raw = con.tile([P, CCH * DIM], F32, name="wv_raw")
        wqT = con.tile([P, P], BF16, name="wqT")              # [C, rdim] chunks at cols kc*64
        wkT = con.tile([P, P], BF16, name="wkT")
        wvT = con.tile([P, CCH * DIM], BF16, name="wvT")      # [C-chunk, c] at cols kc*256
        idf = con.tile([P, P], F32, name="idf")
        idb = con.tile([P, P], BF16, name="idb")
        g_sb = con.tile([P, CCH], F32, name="g_sb")
        b_sb = con.tile([P, CCH], F32, name="b_sb")
        sums = con.tile([P, 4], F32, name="sums")            # [sum_c0, sum_c1, sq_c0, sq_c1]
        part_sum = con.tile([P, 2 * NKC], F32, name="part_sum")  # per-(cc,qchunk) partials
        part_sq = con.tile([P, 2 * NKC], F32, name="part_sq")
        red = con.tile([P, 4], F32, name="red")

        stat_in = dram.tile([P, 4], F32, name="stat_in")
        stat_out = dram.tile([P, 4], F32, name="stat_out")

        # ---- input DMAs (chunked so projections start early) ----
        nc.sync.dma_start(wq_raw[:], wq[:, :])
        nc.sync.dma_start(wk_raw[:], wk[:, :])
        for cc in range(CCH):
            nc.sync.dma_start(wv_raw[:, cc * DIM:(cc + 1) * DIM], wv[cc * P:(cc + 1) * P, :])
        nc.sync.dma_start(g_sb[:], gamma.rearrange("(k p) -> p k", p=P))
        nc.sync.dma_start(b_sb[:], beta.rearrange("(k p) -> p k", p=P))
        for x_sb, x_b, x_dram in ((xkv_sb, xkv_b, xkv), (xq_sb, xq_b, xq)):
            for i in range(NT):
                for cc in range(CCH):
                    s = slice(cc * N + i * NQT, cc * N + (i + 1) * NQT)
                    eng = nc.sync if cc == 0 else nc.gpsimd
                    eng.dma_start(
                        x_sb[:, s], x_dram[cc * P:(cc + 1) * P, i * NQT:(i + 1) * NQT]
                    )
                    nc.vector.tensor_copy(x_b[:, s], x_sb[:, s])

        make_identity(nc, idf[:])
        make_identity(nc, idb[:])
        # ones column of vt (col 256 of each 257-wide tile)
        nc.vector.memset(vt[:].rearrange("p (j c) -> p j c", c=VTW)[:, :, DIM:DIM + 1], 1.0)

        # ---- transpose weights on PE ----
        # wqT/wkT chunk kc: (C 128, rdim 64) at cols kc*64
        for w_raw, wT in ((wq_raw, wqT), (wk_raw, wkT)):
            for kc in range(CCH):
                pst = ps_t.tile([P, P], F32, tag="pst", name="pst")
                nc.tensor.transpose(
                    pst[:, :RDIM], w_raw[:, kc * P:(kc + 1) * P], idf[:RDIM, :RDIM]
                )
                nc.vector.tensor_copy(wT[:, kc * RDIM:(kc + 1) * RDIM], pst[:, :RDIM])
        # wvT chunk kc (C in [kc*128..), c 0..256) from blocks of wv
        for kc in range(CCH):
            for cc in range(CCH):
                pst = ps_t.tile([P, P], F32, tag="pst", name="pst")
                nc.tensor.transpose(
                    pst[:], wv_raw[:, cc * DIM + kc * P: cc * DIM + (kc + 1) * P], idf[:]
                )
                nc.vector.tensor_copy(
                    wvT[:, kc * DIM + cc * P: kc * DIM + (cc + 1) * P], pst[:]
                )

        # ---- compute phases (repeatable for timing amplification) ----
        if LOOP_R > 0:
            with tc.For_i(0, LOOP_R, 1):
                _emit_attention(tc, locals())
            _emit_bn(tc, locals())
        else:
            for _rep in range(REPEAT):
                _emit_compute(tc, locals())



def _emit_compute(tc, env):
    _emit_attention(tc, env)
    _emit_bn(tc, env)


def _emit_attention(tc, env):
    nc = tc.nc
    Exp = mybir.ActivationFunctionType.Exp
    Sqrt = mybir.ActivationFunctionType.Sqrt
    add = mybir.AluOpType.add
    mult = mybir.AluOpType.mult
    subtract = mybir.AluOpType.subtract
    AX = mybir.AxisListType.X
    xq_sb = env["xq_sb"]; xkv_sb = env["xkv_sb"]; xq_b = env["xq_b"]; xkv_b = env["xkv_b"]
    qf = env["qf"]; kf2 = env["kf2"]; vt = env["vt"]
    wqT = env["wqT"]; wkT = env["wkT"]; wvT = env["wvT"]
    idb = env["idb"]; g_sb = env["g_sb"]; b_sb = env["b_sb"]
    sums = env["sums"]; red = env["red"]; stat_in = env["stat_in"]; stat_out = env["stat_out"]
    ps_s = env["ps_s"]; ps_o = env["ps_o"]; ps_t = env["ps_t"]
    sml = env["sml"]; expp = env["expp"]; ctxp = env["ctxp"]
    part_sum = env["part_sum"]; part_sq = env["part_sq"]

    if True:
        # ---- projections ----
        # k: chunk j (j<16) -> kf2[0:64, j*128..]; chunk j+16 -> kf2[64:128, j*128..]
        for t in range(NT if RUN_PROJ else 0):
            psk = ps_s.tile([RDIM, NQT], F32, tag="ps", name="psk")
            for kc in range(CCH):
                nc.tensor.matmul(
                    psk[:], wkT[:, kc * RDIM:(kc + 1) * RDIM],
                    xkv_b[:, kc * N + t * NQT: kc * N + (t + 1) * NQT],
                    start=(kc == 0), stop=(kc == CCH - 1),
                )
            if t < NT // 2:
                nc.vector.tensor_copy(psk_dst_top(kf2, t), psk[:])
            else:
                # bottom half lives at partitions 64..127; engines are
                # lane-locked so stage in SBUF and DMA across partitions
                kstg = sml.tile([RDIM, NQT], BF16, tag="kstg", name="kstg")
                nc.vector.tensor_copy(kstg[:], psk[:])
                nc.sync.dma_start(psk_dst_bot(kf2, t), kstg[:])

        # q: (rdim 64, nq) tiles
        for t in range(NT if RUN_PROJ else 0):
            psq = ps_s.tile([RDIM, NQT], F32, tag="ps", name="psq")
            for kc in range(CCH):
                nc.tensor.matmul(
                    psq[:], wqT[:, kc * RDIM:(kc + 1) * RDIM],
                    xq_b[:, kc * N + t * NQT: kc * N + (t + 1) * NQT],
                    start=(kc == 0), stop=(kc == CCH - 1),
                )
            nc.vector.tensor_copy(qf[0:RDIM, t * NQT:(t + 1) * NQT], psq[:])
        # duplicate q to partitions 64..127 (engines are lane-locked; DMA moves partitions)
        if RUN_PROJ:
            nc.sync.dma_start(qf[RDIM:P, :], qf[0:RDIM, :])

        # v^T: (pix chunk j on partitions, c on free) + ones col
        for j in range(NKC if RUN_PROJ else 0):
            pool_j = ps_o if j % 2 == 0 else ps_s
            psv = pool_j.tile([P, DIM], F32, tag="pso" if j % 2 == 0 else "ps",
                              name="psv")
            for kc in range(CCH):
                nc.tensor.matmul(
                    psv[:], xkv_b[:, kc * N + j * P: kc * N + (j + 1) * P],
                    wvT[:, kc * DIM:(kc + 1) * DIM],
                    start=(kc == 0), stop=(kc == CCH - 1),
                )
            if j % 2 == 0:
                nc.scalar.copy(vt[:, j * VTW: j * VTW + DIM], psv[:])
            else:
                nc.vector.tensor_copy(vt[:, j * VTW: j * VTW + DIM], psv[:])

        # ---- main attention loop (software-pipelined over query tiles) ----
        # scores/exp for tile t are interleaved at fine grain with attn@v of
        # tile t-1 so the PE's in-order queue always has ready work while the
        # ACT engine drains exp chunks (2 score-pairs, then 16 attn@v mms).
        exp_tiles = {}

        def emit_pair(t, jp):
            exp_t = exp_tiles[t]
            psW = ps_s.tile([P, 2 * NQT], F32, tag="ps", name="psW")
            nc.tensor.matmul(
                psW[:, 0:NQT], kf2[0:RDIM, jp * P:(jp + 1) * P],
                qf[0:RDIM, t * NQT:(t + 1) * NQT],
                start=True, stop=True,
            )
            nc.tensor.matmul(
                psW[:, NQT:2 * NQT], kf2[RDIM:P, jp * P:(jp + 1) * P],
                qf[RDIM:P, t * NQT:(t + 1) * NQT],
                start=True, stop=True,
            )
            # one wide exp: halves land at chunk jp and jp+NPAIR slices
            dst = exp_t[:].rearrange("p (g c) -> p g c", g=2)[
                :, :, jp * NQT:(jp + 1) * NQT]
            srcv = psW[:].rearrange("p (g c) -> p g c", g=2)
            nc.scalar.activation(dst, srcv, Exp, scale=SCALE)

        def emit_attnv_chunk_mms(t, m, pso, j0, j1):
            exp_t = exp_tiles[t]
            for j in range(j0, j1):
                nc.tensor.matmul(
                    pso[:], exp_t[:, j * NQT + m * P: j * NQT + (m + 1) * P],
                    vt[:, j * VTW:(j + 1) * VTW],
                    start=(j == 0), stop=(j == NKC - 1),
                )

        def emit_attnv_chunk_tail(t, m, pso):
            rec = sml.tile([P, 1], F32, tag="rec", name="rec")
            nc.vector.reciprocal(rec[:], pso[:, DIM:DIM + 1])
            ctxn = ctxp.tile([P, DIM], BF16, tag="ctxn", name="ctxn")
            nc.vector.tensor_scalar_mul(ctxn[:], pso[:, 0:DIM], rec[:])
            nq0 = t * NQT + m * P
            qi = t * NQC + m
            for cc in range(CCH):
                pstt = ps_t.tile([P, P], BF16, tag="pst", name="pstt")
                nc.tensor.transpose(pstt[:], ctxn[:, cc * P:(cc + 1) * P], idb[:])
                ys = xq_sb[:, cc * N + nq0: cc * N + nq0 + P]
                nc.vector.tensor_tensor(out=ys, in0=ys, in1=pstt[:], op=add)
                # streaming BN partials: sum on DVE, sum-of-squares on ACT
                nc.vector.reduce_sum(
                    part_sum[:, cc * NKC + qi: cc * NKC + qi + 1], ys, axis=AX
                )
                sq_scr = sml.tile([P, P], F32, tag="sq_scr", name="sq_scr")
                nc.vector.scalar_tensor_tensor(
                    out=sq_scr[:], in0=ys, scalar=1.0, in1=ys,
                    op0=mult, op1=mult,
                    accum_out=part_sq[:, cc * NKC + qi: cc * NKC + qi + 1],
                )

        NH = NKC // 2  # mm2 j-chunks per half group
        for t in range(NT_RUN + 1):
            cur = t if t < NT_RUN else None
            prev = t - 1 if t >= 1 else None
            if cur is not None:
                exp_tiles[cur] = expp.tile(
                    [P, NKC * NQT], BF16, tag="exp", name="exp_t"
                )
            for m in range(NQC):
                pso = (ps_o.tile([P, VTW], F32, tag="pso", name="pso")
                       if prev is not None else None)
                for half in range(2):
                    if cur is not None:
                        for q in range(2):
                            emit_pair(cur, m * 4 + half * 2 + q)
                    if prev is not None:
                        emit_attnv_chunk_mms(prev, m, pso, half * NH, (half + 1) * NH)
                if prev is not None:
                    emit_attnv_chunk_tail(prev, m, pso)
            if prev is not None:
                exp_tiles.pop(prev)



def _emit_bn(tc, env):
    nc = tc.nc
    Sqrt = mybir.ActivationFunctionType.Sqrt
    add = mybir.AluOpType.add
    mult = mybir.AluOpType.mult
    subtract = mybir.AluOpType.subtract
    AX = mybir.AxisListType.X
    xq_sb = env["xq_sb"]; xkv_sb = env["xkv_sb"]
    g_sb = env["g_sb"]; b_sb = env["b_sb"]
    sums = env["sums"]; red = env["red"]; stat_in = env["stat_in"]; stat_out = env["stat_out"]
    sml = env["sml"]

    part_sum = env["part_sum"]; part_sq = env["part_sq"]

    if True:
        # ---- BN stats from streamed partials ----
        for cc in range(CCH):
            nc.vector.reduce_sum(
                sums[:, cc:cc + 1], part_sum[:, cc * NKC:(cc + 1) * NKC], axis=AX
            )
            nc.vector.reduce_sum(
                sums[:, 2 + cc:3 + cc], part_sq[:, cc * NKC:(cc + 1) * NKC], axis=AX
            )
        nc.gpsimd.dma_start(stat_in[:], sums[:])
        if USE_COLLECTIVE:
            nc.gpsimd.collective_compute(
                "AllReduce", add,
                ins=[stat_in[:]], outs=[stat_out[:]],
                replica_groups=[[0, 1, 2, 3], [4, 5, 6, 7]],
            )
        else:
            nc.gpsimd.dma_start(stat_out[:], stat_in[:])
        nc.gpsimd.dma_start(red[:], stat_out[:])

        meanv = sml.tile([P, CCH], F32, tag="meanv", name="meanv")
        es2 = sml.tile([P, CCH], F32, tag="es2", name="es2")
        varp = sml.tile([P, CCH], F32, tag="varp", name="varp")
        rstd = sml.tile([P, CCH], F32, tag="rstd", name="rstd")
        scl = sml.tile([P, CCH], F32, tag="scl", name="scl")
        sh = sml.tile([P, CCH], F32, tag="sh", name="sh")
        nc.vector.tensor_scalar_mul(meanv[:], red[:, 0:CCH], 1.0 / NSTAT)
        nc.vector.tensor_scalar_mul(es2[:], red[:, CCH:2 * CCH], 1.0 / NSTAT)
        nc.vector.tensor_tensor(out=varp[:], in0=meanv[:], in1=meanv[:], op=mult)
        nc.vector.tensor_tensor(out=varp[:], in0=es2[:], in1=varp[:], op=subtract)
        nc.vector.tensor_scalar_add(varp[:], varp[:], EPS)
        nc.scalar.activation(rstd[:], varp[:], Sqrt)
        nc.vector.reciprocal(rstd[:], rstd[:])
        nc.vector.tensor_tensor(out=scl[:], in0=g_sb[:], in1=rstd[:], op=mult)
        nc.vector.tensor_tensor(out=sh[:], in0=meanv[:], in1=scl[:], op=mult)
        nc.vector.tensor_tensor(out=sh[:], in0=b_sb[:], in1=sh[:], op=subtract)

        out = env["out"]
        for cc in range(CCH):
            for i in range(NT):
                s = slice(cc * N + i * NQT, cc * N + (i + 1) * NQT)
                nc.vector.tensor_scalar(
                    out=xq_sb[:, s], in0=xq_sb[:, s],
                    scalar1=scl[:, cc:cc + 1], scalar2=sh[:, cc:cc + 1],
                    op0=mult, op1=add,
                )
                (nc.sync if cc == 0 else nc.gpsimd).dma_start(
                    out[cc * P:(cc + 1) * P, i * NQT:(i + 1) * NQT], xq_sb[:, s]
                )


def psk_dst_top(kf2, t):
    # tiles t<4 hold chunks 4t..4t+3 -> top half, cols (4t)*128..
    c0 = t * NQT
    return kf2[0:RDIM, c0:c0 + NQT]


def psk_dst_bot(kf2, t):
    # tiles t>=4 hold chunks 4t..4t+3 = (j+16 for j=4t-16..) -> bottom half
    c0 = (t - NT // 2) * NQT
    return kf2[RDIM:P, c0:c0 + NQT]


def get_program():
    if "nc" not in _CACHE:
        _CACHE["nc"] = _build_program()
    return _CACHE["nc"]


def make_in_maps(wli_feat, nbi_aligned, w_q_wli, w_k_nbi, w_v_nbi,
                 w_q_nbi, w_k_wli, w_v_wli, gamma_wli, beta_wli,
                 gamma_nbi, beta_nbi):
    """Per-core inputs: cores 0..3 = (batch b, wli), cores 4..7 = (batch b, nbi),
    so modality groups are {0,1,2,3} (wli) and {4,5,6,7} (nbi)."""

    def f(x):
        return np.ascontiguousarray(np.asarray(x, dtype=np.float32))

    in_maps = []
    for b in range(B):
        in_maps.append({
            "xq": f(wli_feat[b]).reshape(DIM, N),
            "xkv": f(nbi_aligned[b]).reshape(DIM, N),
            "wq": f(w_q_wli), "wk": f(w_k_nbi), "wv": f(w_v_nbi),
            "gamma": f(gamma_wli), "beta": f(beta_wli),
        })
    for b in range(B):
        in_maps.append({
            "xq": f(nbi_aligned[b]).reshape(DIM, N),
            "xkv": f(wli_feat[b]).reshape(DIM, N),
            "wq": f(w_q_nbi), "wk": f(w_k_wli), "wv": f(w_v_wli),
            "gamma": f(gamma_nbi), "beta": f(beta_nbi),
        })
    return in_maps


def assemble_outputs(results):
    enh_wli = np.empty((B, DIM, H, W), np.float32)
    enh_nbi = np.empty((B, DIM, H, W), np.float32)
    for b in range(B):
        enh_wli[b] = results[b]["out"].reshape(DIM, H, W)
        enh_nbi[b] = results[B + b]["out"].reshape(DIM, H, W)
    return enh_wli, enh_nbi


def kernel(**inputs):
    nc = get_program()
    in_maps = make_in_maps(**inputs)
    res = run_bass_kernel_spmd(nc, in_maps, list(range(NCORES)))
    return assemble_outputs(res.results)

